# revision 15
# baseline (speedup 1.0000x reference)
"""Local (sliding-window w=2) attention, B=4 S=2048 H=1024, on 8 trn2 cores.

Strategy: sequence-parallel. Each core owns half of one batch's sequence
(1024 tokens) plus a 2-token halo on each side (ext = 1028 tokens).

Q/K projections run in fp8(e4m3) with DoubleRow perf mode (2 contraction
rows packed per PE cell -> ~1.5-2x fp16 matmul throughput). x is scaled
by 32 and W by 2048 on the host; the PSUM result is unscaled + biased on
DVE during evacuation to fp16 Q^T/K^T. V projection and P@V stay fp16
(fp8 V fails the 2e-2 rel-err budget; fp8 Q/K lands at ~1.55e-2 measured
on the real inputs). Output is written fp16 and widened on the host.

DMA: the scalar HWDGE ring stripes across ~13 SDMA engines (~300 GB/s)
while the sync ring only gets ~3 (~75 GB/s), so all big input streams go
on the scalar ring in consumption order; small constants go on sync.
Block outputs alternate sync/scalar rings so the last block's store is
never queued behind 8 earlier stores (the baseline lost ~15 us to a
single-ring output tail at ~78 GB/s).

Per core: 9 q-blocks of 123 queries: band scores (window 127), masked
softmax (ACT exp + fused row-sum), P transpose on PE, P@V with a ones
column folding in the V bias, fp16 out, DMA. Blocks are software-
pipelined one stage so softmax hides under the next block's V projection.
"""

import os
import sys

sys.path.insert(0, "/opt/trn_rl_repo")

import ml_dtypes
import numpy as np

import concourse.bass as bass  # noqa: F401  (bass must import before tile)
import concourse.mybir as mybir
import concourse.tile as tile
from concourse import bacc
from concourse.bass_utils import run_bass_kernel_spmd

F32 = mybir.dt.float32
F16 = mybir.dt.float16
F8 = mybir.dt.float8e4
E4NP = ml_dtypes.float8_e4m3
DR = mybir.MatmulPerfMode.DoubleRow

B, S, H = 4, 2048, 1024
WCTX = 2
NCORES = 8
SHARD = S // 2  # tokens per core
EXT = SHARD + 2 * WCTX  # 1028
TH = 528  # fp8 x token-half width (514/516 used), 16B-aligned hc stride
P = 128
QB = 123  # queries per attention block
WIN = QB + 2 * WCTX  # 127 = key window per block
NBLK = (SHARD + QB - 1) // QB  # 9
HC = H // P  # 8 feature chunks
SCALE = 1.0 / np.sqrt(np.float32(H))
SX = 32.0  # host scale on x before fp8
SW = 2048.0  # host scale on W before fp8
UNSCALE = 1.0 / (SX * SW)

_prog_cache = {}


def _build_program():
    nc = bacc.Bacc("TRN2", target_bir_lowering=False, debug=False)
    # x8: token halves th0 = ext [0,514), th1 = ext [512,1028), each padded
    # to TH=528 cols so DoubleRow's hc stride is 16B-aligned and each DMA
    # half is one contiguous 4224B run per partition.
    x8_d = nc.dram_tensor("x8", [P, 2 * HC * TH], F8, kind="ExternalInput").ap()
    x16_d = nc.dram_tensor("x16", [P, HC * EXT], F16, kind="ExternalInput").ap()
    # wq/wk: j-halves jh, [p, jh, hc, 512] -> contiguous 4096B per partition
    wq_d = nc.dram_tensor("wq", [P, 2 * HC * 512], F8, kind="ExternalInput").ap()
    wk_d = nc.dram_tensor("wk", [P, 2 * HC * 512], F8, kind="ExternalInput").ap()
    wv_d = nc.dram_tensor("wv", [P, HC * H], F16, kind="ExternalInput").ap()
    bq_d = nc.dram_tensor("bq_c", [P, HC], F32, kind="ExternalInput").ap()
    bk_d = nc.dram_tensor("bk_c", [P, HC], F32, kind="ExternalInput").ap()
    bv_d = nc.dram_tensor("bv_r", [P, H], F16, kind="ExternalInput").ap()
    id_d = nc.dram_tensor("ident", [P, P], F16, kind="ExternalInput").ap()
    mk_d = nc.dram_tensor("mask", [QB, NBLK * WIN], F32, kind="ExternalInput").ap()
    out_d = nc.dram_tensor("out", [SHARD, H], F16, kind="ExternalOutput").ap()

    x8_r = x8_d.rearrange("p (th hc t) -> p th hc t", th=2, hc=HC)
    x16_r = x16_d.rearrange("p (hc t) -> p hc t", hc=HC)
    wq_r = wq_d.rearrange("p (jh hc j) -> p jh hc j", jh=2, hc=HC)
    wk_r = wk_d.rearrange("p (jh hc j) -> p jh hc j", jh=2, hc=HC)
    wv_r = wv_d.rearrange("p (hc j) -> p hc j", hc=HC)
    mk_r = mk_d.rearrange("q (b c) -> q b c", b=NBLK)

    with tile.TileContext(nc) as tc:
        with (
            tc.tile_pool(name="persist", bufs=1) as pers,
            tc.tile_pool(name="vpool", bufs=3) as vpool,
            tc.tile_pool(name="spool", bufs=2) as spool,
            tc.tile_pool(name="opool", bufs=4) as opool,
            tc.tile_pool(name="pproj", bufs=3, space="PSUM") as pproj,
            tc.tile_pool(name="patt", bufs=2, space="PSUM") as patt,
            tc.tile_pool(name="pout", bufs=1, space="PSUM") as pout,
            tc.tile_pool(name="ptp", bufs=1, space="PSUM") as ptp,
        ):
            # ---- gpsimd SWDGE wakes ~3us before the HWDGE rings: tiny
            # consts + the first halves of x8/wq so the PE can start early ----
            ident = pers.tile([P, P], F16)
            nc.gpsimd.dma_start(ident[:], id_d)
            bqc = pers.tile([P, HC], F32)
            nc.gpsimd.dma_start(bqc[:], bq_d)
            bkc = pers.tile([P, HC], F32)
            nc.gpsimd.dma_start(bkc[:], bk_d)
            x8_sb = pers.tile([P, 2, HC, TH], F8)
            wq_sb = pers.tile([P, 2, HC, 512], F8)
            wk_sb = pers.tile([P, 2, HC, 512], F8)
            x16_sb = pers.tile([P, HC, EXT], F16)
            wv_sb = pers.tile([P, HC, H], F16)
            maskt = pers.tile([QB, NBLK, WIN], F32)
            nc.gpsimd.dma_start(x8_sb[:, 0, 4:], x8_r[:, 0, 4:])
            nc.gpsimd.dma_start(wq_sb[:, 0, 4:], wq_r[:, 0, 4:])

            # ---- scalar ring: big streams in consumption order, each
            # instruction one contiguous run per partition ----
            nc.scalar.dma_start(x8_sb[:, 0, :4], x8_r[:, 0, :4])
            nc.scalar.dma_start(wq_sb[:, 0, :4], wq_r[:, 0, :4])
            nc.scalar.dma_start(wq_sb[:, 1], wq_r[:, 1])
            nc.scalar.dma_start(x8_sb[:, 1], x8_r[:, 1])
            nc.scalar.dma_start(wk_sb[:, 0], wk_r[:, 0])
            nc.scalar.dma_start(wk_sb[:, 1], wk_r[:, 1])
            nc.scalar.dma_start(x16_sb[:, :4], x16_r[:, :4])
            nc.scalar.dma_start(x16_sb[:, 4:], x16_r[:, 4:])
            nc.scalar.dma_start(wv_sb[:, :4], wv_r[:, :4])
            nc.scalar.dma_start(wv_sb[:, 4:], wv_r[:, 4:])

            # ---- sync ring: attention-phase consts ----
            bvr = pers.tile([P, H], F16)
            nc.sync.dma_start(bvr[:], bv_d)
            nc.sync.dma_start(maskt[:], mk_r)

            # ---- PE warm-up: ramp the HAM clock gate while DMAs land ----
            with tc.high_priority():
                for _ in range(6):
                    pwu = ptp.tile([P, P], F16, tag="pt")
                    nc.tensor.transpose(pwu[:], ident[:], ident[:])

            # ---- Q^T projection: fp8 DoubleRow, owned tokens ext [2, 1026) ----
            # t-chunk 0 = th0 cols [2,514), t-chunk 1 = th1 cols [2,514)
            qt_sb = pers.tile([P, HC, SHARD], F16)
            for t in range(2):
                for jc in range(HC):
                    jh, jj = divmod(jc, 4)
                    ps = pproj.tile([P, 512], F32, tag="proj")
                    for c in range(4):
                        nc.tensor.matmul(
                            ps[:],
                            wq_sb[:, jh, 2 * c : 2 * c + 2, jj * P : (jj + 1) * P],
                            x8_sb[:, t, 2 * c : 2 * c + 2, 2:514],
                            start=(c == 0),
                            stop=(c == 3),
                            perf_mode=DR,
                        )
                    nc.vector.tensor_scalar(
                        qt_sb[:, jc, 512 * t : 512 * (t + 1)],
                        ps[:],
                        UNSCALE,
                        bqc[:, jc : jc + 1],
                        mybir.AluOpType.mult,
                        mybir.AluOpType.add,
                    )

            # ---- K^T projection: fp8, ext tokens [0, 1024) then halo ----
            # t-chunk 0 = th0 cols [0,512), t-chunk 1 = th1 cols [0,512)
            kt_sb = pers.tile([P, HC, EXT], F16)
            for t in range(2):
                for jc in range(HC):
                    jh, jj = divmod(jc, 4)
                    ps = pproj.tile([P, 512], F32, tag="proj")
                    for c in range(4):
                        nc.tensor.matmul(
                            ps[:],
                            wk_sb[:, jh, 2 * c : 2 * c + 2, jj * P : (jj + 1) * P],
                            x8_sb[:, t, 2 * c : 2 * c + 2, 0:512],
                            start=(c == 0),
                            stop=(c == 3),
                            perf_mode=DR,
                        )
                    nc.vector.tensor_scalar(
                        kt_sb[:, jc, 512 * t : 512 * (t + 1)],
                        ps[:],
                        UNSCALE,
                        bkc[:, jc : jc + 1],
                        mybir.AluOpType.mult,
                        mybir.AluOpType.add,
                    )
            def emit_k_halo():
                """K^T for halo tokens [1024, 1028) = th1 cols [512, 516):
                plain fp8 matmuls (DoubleRow loses at FD=4). Only block 8's
                scores need these, so this is emitted mid-attention where
                the 64 tiny matmuls fill a PE bubble instead of delaying
                the first V projection."""
                for jc in range(HC):
                    jh, jj = divmod(jc, 4)
                    ps = pproj.tile([P, 512], F32, tag="proj")
                    for hc in range(HC):
                        nc.tensor.matmul(
                            ps[:, :4],
                            wk_sb[:, jh, hc, jj * P : (jj + 1) * P],
                            x8_sb[:, 1, hc, 512:516],
                            start=(hc == 0),
                            stop=(hc == HC - 1),
                        )
                    nc.vector.tensor_scalar(
                        kt_sb[:, jc, 1024:1028],
                        ps[:, :4],
                        UNSCALE,
                        bkc[:, jc : jc + 1],
                        mybir.AluOpType.mult,
                        mybir.AluOpType.add,
                    )

            # ---- attention blocks, software-pipelined by one stage ----
            def blk_geom(b):
                q0 = QB * b
                qb = min(QB, SHARD - q0)
                return q0, qb, qb + 2 * WCTX

            def emit_v(b):
                """V for block b's window, token-major [w, H], fp16."""
                q0, qb, w = blk_geom(b)
                vb = vpool.tile([P, H], F16, tag="vblk")
                for n in range(2):
                    psv = pproj.tile([P, 512], F32, tag="proj")
                    for hc in range(HC):
                        nc.tensor.matmul(
                            psv[:w, :],
                            x16_sb[:, hc, q0 : q0 + w],
                            wv_sb[:, hc, 512 * n : 512 * (n + 1)],
                            start=(hc == 0),
                            stop=(hc == HC - 1),
                        )
                    nc.scalar.copy(vb[:w, 512 * n : 512 * (n + 1)], psv[:w, :])
                nc.gpsimd.dma_start(vb[w : w + 1, :], bvr[w : w + 1, :])
                return vb

            def emit_scores_softmax(b):
                """Scores + masked softmax; returns normalized P tile (fp16)."""
                q0, qb, w = blk_geom(b)
                pss = patt.tile([QB, WIN], F32, tag="ps")
                for jc in range(HC):
                    nc.tensor.matmul(
                        pss[:qb, :w],
                        qt_sb[:, jc, q0 : q0 + qb],
                        kt_sb[:, jc, q0 : q0 + w],
                        start=(jc == 0),
                        stop=(jc == HC - 1),
                    )
                sm = spool.tile([QB, WIN], F32, tag="sm")
                nc.vector.tensor_tensor(
                    sm[:qb, :w], pss[:qb, :w], maskt[:qb, b, :w], op=mybir.AluOpType.add
                )
                pexp = spool.tile([QB, WIN], F32, tag="pexp")
                rsum = spool.tile([QB, 1], F32, tag="rsum")
                nc.scalar.activation(
                    pexp[:qb, :w],
                    sm[:qb, :w],
                    mybir.ActivationFunctionType.Exp,
                    bias=0.0,
                    scale=float(SCALE),
                    accum_out=rsum[:qb],
                )
                rcp = spool.tile([QB, 1], F32, tag="rcp")
                nc.vector.reciprocal(rcp[:qb], rsum[:qb])
                pn = spool.tile([QB, WIN + 1], F16, tag="pn")
                nc.vector.tensor_scalar_mul(pn[:qb, :w], pexp[:qb, :w], rcp[:qb])
                nc.vector.memset(pn[:qb, w : w + 1], 1.0)
                return pn

            def emit_pv_out(b, pn, vb):
                """Transpose P, P@V, fp16 out, DMA on alternating rings."""
                q0, qb, w = blk_geom(b)
                pst = ptp.tile([WIN + 1, QB], F16, tag="pt")
                nc.tensor.transpose(pst[: w + 1, :qb], pn[:qb, : w + 1], ident[:qb, :qb])
                pts = spool.tile([WIN + 1, QB], F16, tag="pts")
                nc.vector.tensor_copy(pts[: w + 1, :qb], pst[: w + 1, :qb])
                pso = pout.tile([QB, H], F32, tag="po")
                ob = opool.tile([QB, H], F16, tag="ob")
                for n in range(2):
                    nc.tensor.matmul(
                        pso[:qb, 512 * n : 512 * (n + 1)],
                        pts[: w + 1, :qb],
                        vb[: w + 1, 512 * n : 512 * (n + 1)],
                        start=True,
                        stop=True,
                    )
                    eng = nc.vector.tensor_copy if n == 0 else nc.scalar.copy
                    eng(
                        ob[:qb, 512 * n : 512 * (n + 1)],
                        pso[:qb, 512 * n : 512 * (n + 1)],
                    )
                # 3 parallel store instructions: single-instruction DMAs only
                # stripe ~3 SDMA engines, so split by partition range across
                # both HWDGE rings
                t1, t2 = qb // 3, 2 * qb // 3
                nc.scalar.dma_start(out_d[q0 : q0 + t1, :], ob[:t1, :])
                nc.scalar.dma_start(out_d[q0 + t1 : q0 + t2, :], ob[t1:t2, :])
                nc.sync.dma_start(out_d[q0 + t2 : q0 + qb, :], ob[t2:qb, :])

            # depth-2 pipeline: V and scores/softmax of blocks b+1, b+2 hide
            # under block b's transpose/PV on the PE
            stage = []
            for b in range(min(2, NBLK)):
                stage.append((emit_v(b), emit_scores_softmax(b)))
            for b in range(NBLK):
                vb, pn = stage[b]
                emit_pv_out(b, pn, vb)
                if b == 1:
                    emit_k_halo()
                if b + 2 < NBLK:
                    stage.append((emit_v(b + 2), emit_scores_softmax(b + 2)))

    nc.compile()
    return nc


def _build_mask(h: int) -> np.ndarray:
    mask = np.full((QB, NBLK, WIN), -1e30, dtype=np.float32)
    r = np.arange(QB)[:, None]
    c = np.arange(WIN)[None, :]
    band = (c - r >= 0) & (c - r <= 2 * WCTX)
    for b in range(NBLK):
        q0 = QB * b
        qb = min(QB, SHARD - q0)
        gk = h * SHARD + q0 + c - WCTX  # global key token index
        valid = band & (gk >= 0) & (gk < S) & (r < qb) & (c < qb + 2 * WCTX)
        mask[:, b, :] = np.where(valid, np.float32(0.0), np.float32(-1e30))
    return mask.reshape(QB, NBLK * WIN)


def _pack_rows(a: np.ndarray) -> np.ndarray:
    """[H, C] row-major -> [P, HC*C]: partition p line = rows p, 128+p, ..."""
    C = a.shape[1]
    return np.ascontiguousarray(
        a.reshape(HC, P, C).transpose(1, 0, 2).reshape(P, HC * C)
    )


def kernel(sequence_output, Wq, bq, Wk, bk, Wv, bv):
    x = np.asarray(sequence_output, dtype=np.float32)
    Wq = np.asarray(Wq, dtype=np.float32)
    Wk = np.asarray(Wk, dtype=np.float32)
    Wv = np.asarray(Wv, dtype=np.float32)
    bq = np.asarray(bq, dtype=np.float32)
    bk = np.asarray(bk, dtype=np.float32)
    bv = np.asarray(bv, dtype=np.float32)

    if "nc" not in _prog_cache:
        _prog_cache["nc"] = _build_program()
    nc = _prog_cache["nc"]

    def _pack_w8(W):
        """[H, H] -> [P, jh*hc*512] with (p, jh, hc, j) order, fp8."""
        a = (W * SW).astype(E4NP).view(np.uint8)  # [H=hc*P rows, H=jh*512 cols]
        a = a.reshape(HC, P, 2, 512).transpose(1, 2, 0, 3)  # p, jh, hc, 512
        return np.ascontiguousarray(a.reshape(P, 2 * HC * 512)).view(E4NP)

    wq8 = _pack_w8(Wq)
    wk8 = _pack_w8(Wk)
    wv_h = _pack_rows(Wv.astype(np.float16))
    bq_c = np.ascontiguousarray(bq.reshape(HC, P).T)
    bk_c = np.ascontiguousarray(bk.reshape(HC, P).T)
    bv_r = np.ascontiguousarray(np.broadcast_to(bv, (P, H))).astype(np.float16)
    ident = np.eye(P, dtype=np.float16)
    masks = [_build_mask(0), _build_mask(1)]

    # pad each sequence with WCTX zero rows on both ends, slice ext windows
    xp = np.zeros((B, S + 2 * WCTX, H), dtype=np.float32)
    xp[:, WCTX : WCTX + S] = x

    in_maps = []
    for c in range(NCORES):
        bidx, h = divmod(c, 2)
        ext = xp[bidx, h * SHARD : h * SHARD + EXT]  # [EXT, H]
        xt = np.ascontiguousarray(ext.T)  # [H, EXT] f32
        x16 = _pack_rows(xt.astype(np.float16))
        # fp8 token halves: th0 = ext [0,514), th1 = ext [512,1028), pad TH=528
        x8q = (xt * SX).astype(E4NP)
        x8e = np.zeros((H, 2, TH), dtype=E4NP)
        x8e[:, 0, :514] = x8q[:, 0:514]
        x8e[:, 1, :516] = x8q[:, 512:1028]
        a = x8e.view(np.uint8).reshape(HC, P, 2, TH).transpose(1, 2, 0, 3)
        x8 = np.ascontiguousarray(a.reshape(P, 2 * HC * TH)).view(E4NP)
        in_maps.append(
            {
                "x8": x8,
                "x16": x16,
                "wq": wq8,
                "wk": wk8,
                "wv": wv_h,
                "bq_c": bq_c,
                "bk_c": bk_c,
                "bv_r": bv_r,
                "ident": ident,
                "mask": masks[h],
            }
        )

    trace = bool(int(os.environ.get("LK_TRACE", "0")))
    res = run_bass_kernel_spmd(
        nc,
        in_maps,
        core_ids=list(range(NCORES)),
        trace=trace,
        trace_cores=list(range(NCORES)) if trace else None,
    )
    _prog_cache["last_results"] = res

    out = np.empty((B, S, H), dtype=np.float32)
    for c in range(NCORES):
        bidx, h = divmod(c, 2)
        out[bidx, h * SHARD : (h + 1) * SHARD] = res.results[c]["out"].astype(
            np.float32
        )
    return out
